# revision 2
# baseline (speedup 1.0000x reference)
"""MetaCA Trainium2 kernel: 8-core data-parallel (one batch row per core).

Layout: cells kept resident in SBUF as [D=128 partitions, 1+T+1 tokens]
(one halo column on each side for the +-1 rolls). Per evolve iteration,
per 1024-token macro-tile:
  GEMM1  (f32r): h_pre[2048 feat, tok] = W1^T @ [x; roll+1; roll-1]
                 (rolls are free: +-1 column offsets into the cells buffer)
  GELU   (ACT) : one [128, 2048] op per rule-pair PSUM tile
  GEMM2  (f32r): y2[d, tok] per rule, K=256
  TANH   (ACT) : one [128, 2048] op per 2 rules
  R-sum  (DVE) : acc += (1-alpha)*w_r * t_r  (scalar_tensor_tensor FMA)
  blend  (DVE) : new = alpha*old + acc
Selector MLPs (rule weights / n_evolve / alpha) are tiny and computed on
host in float64; they only depend on c_state. Final LayerNorm is done
after a PE transpose to [token, D] tiles, with rsqrt = reciprocal(sqrt)
plus one Newton step (the HW sqrt table is low-precision).
"""

import numpy as np
from contextlib import ExitStack

import concourse.bass as bass
import concourse.bacc as bacc
import concourse.mybir as mybir
from concourse.tile import TileContext
from concourse.bass_utils import run_bass_kernel_spmd
from concourse.masks import make_identity

B, T, D, R = 8, 4096, 128, 8
H2 = 2 * D            # 256 hidden per rule
M1 = R * H2           # 2048 GEMM1 output features
LN_EPS = 1e-5
TT = 1024             # macro token tile
NMT = T // TT         # 4 macro tiles
F32 = mybir.dt.float32
F32R = mybir.dt.float32r
F16 = mybir.dt.float16
AF = mybir.ActivationFunctionType
OP = mybir.AluOpType


def _gelu64(x):
    from scipy.special import erf
    return 0.5 * x * (1.0 + erf(x / np.sqrt(2.0)))


def _softmax64(v):
    e = np.exp(v - v.max())
    return e / e.sum()


def _selectors(inputs):
    f = lambda k: np.asarray(inputs[k], np.float64)
    c = f("c_state")

    def mlp(p):
        return _gelu64(c @ f(p + "_W1") + f(p + "_b1")) @ f(p + "_W2") + f(p + "_b2")

    rw = _softmax64(mlp("rsel"))
    sw = _softmax64(mlp("ssel"))
    n_soft = float((sw * np.arange(2.0, 9.0)).sum())
    n_evolve = max(2, min(8, int(n_soft + 0.5)))
    alpha = float(0.1 + 0.8 / (1.0 + np.exp(-mlp("asel")[0])))
    return [float(w) for w in rw], alpha, n_evolve


def build_nc(n_evolve, alpha, rule_w, mm_dt=F16, apply_gb=False):
    nc = bacc.Bacc("TRN2", target_bir_lowering=False, debug=False)
    x_d = nc.declare_dram_parameter("x", [T, D], F32, isOutput=False)
    w1_d = nc.declare_dram_parameter("w1", [128, 3 * M1], mm_dt, isOutput=False)
    w2_d = nc.declare_dram_parameter("w2", [128, 2 * R * D], mm_dt, isOutput=False)
    if apply_gb:
        gb_d = nc.declare_dram_parameter("gb", [2, 128, D], F32, isOutput=False)
    y_d = nc.declare_dram_parameter("y", [T, D], F32, isOutput=True)

    wp = [w * (1.0 - alpha) for w in rule_w]   # fold (1-alpha) into rule weights

    with ExitStack() as ctx:
        tc = ctx.enter_context(TileContext(nc))
        cpool = ctx.enter_context(tc.tile_pool(name="const", bufs=1))
        cellp = ctx.enter_context(tc.tile_pool(name="cells", bufs=1))
        wpool = ctx.enter_context(tc.tile_pool(name="work", bufs=3))
        apool = ctx.enter_context(tc.tile_pool(name="accp", bufs=2))
        lnp = ctx.enter_context(tc.tile_pool(name="ln", bufs=2))
        ppool = ctx.enter_context(tc.tile_pool(name="psum", bufs=2, space="PSUM"))

        w1_sb = cpool.tile([128, 3 * M1], mm_dt, tag="w1")
        nc.sync.dma_start(w1_sb[:], w1_d[:])
        w2_sb = cpool.tile([128, 2 * R * D], mm_dt, tag="w2")
        nc.sync.dma_start(w2_sb[:], w2_d[:])
        if apply_gb:
            gb_sb = cpool.tile([128, 2 * D], F32, tag="gb")
            for k in range(2):
                nc.sync.dma_start(gb_sb[:, k * D:(k + 1) * D], gb_d[k])
        ident = cpool.tile([128, 128], F32, tag="ident")
        make_identity(nc, ident[:])

        def absorb_mm(ps, dep_ap):
            # Sacrificial normal matmul into ps[:, 0:128]: transpose matmuls
            # have a single sync-wait slot (S3_LW), so absorb the slot-WAR
            # and cross-engine waits here (normal matmuls carry 2 waits).
            nc.tensor.matmul(ps[:, 0:128], dep_ap, dep_ap, start=True, stop=True)

        bufA = cellp.tile([128, T + 2], F32, tag="bufA")
        bufB = cellp.tile([128, T + 2], F32, tag="bufB")
        bufA16 = cellp.tile([128, T + 2], mm_dt, tag="bufA16")
        bufB16 = cellp.tile([128, T + 2], mm_dt, tag="bufB16")

        def mm(out, lhsT, rhs, start, stop):
            nc.tensor.matmul(out, lhsT, rhs, start=start, stop=stop)

        # ---- load input [T, D] and transpose into bufA [D, 1+T+1] ----
        for half in range(2):
            ps = ppool.tile([128, 2048], F32, tag="mm")
            absorb_mm(ps, ident[:])
            if half == 0:
                # absorb the weight-DMA waits so the first GEMM matmuls
                # stay within the 2-wait budget
                absorb_mm(ps, w1_sb[:, 0:128])
                absorb_mm(ps, w2_sb[:, 0:128])
            xt = wpool.tile([128, 2048], F32, tag="xin")
            src = x_d[half * 2048:(half + 1) * 2048, :].rearrange(
                "(j p) d -> p j d", p=128)
            nc.sync.dma_start(xt[:].rearrange("p (j d) -> p j d", j=16), src)
            for j in range(16):
                nc.tensor.transpose(ps[:, j * 128:(j + 1) * 128],
                                    xt[:, j * 128:(j + 1) * 128], ident[:])
            nc.vector.tensor_copy(bufA[:, 1 + half * 2048: 1 + (half + 1) * 2048], ps[:])
            nc.scalar.copy(bufA16[:, 1 + half * 2048: 1 + (half + 1) * 2048], ps[:])
        nc.vector.tensor_copy(bufA16[:, 0:1], bufA16[:, T:T + 1])
        nc.vector.tensor_copy(bufA16[:, T + 1:T + 2], bufA16[:, 1:2])

        # ---- evolve iterations ----
        cur, nxt = bufA, bufB
        cur16, nxt16 = bufA16, bufB16
        for it in range(n_evolve):
            for mt in range(NMT):
                t0 = mt * TT
                h2 = {}
                acc = None
                for r in range(R):
                    # GEMM1 for rule r: PSUM [128, 2048] = (m=2r | m=2r+1) x 1024 tok
                    ps = ppool.tile([128, 2048], F32, tag="mm")
                    for hm in range(2):
                        m = 2 * r + hm
                        for n in range(2):
                            outap = ps[:, hm * TT + n * 512: hm * TT + (n + 1) * 512]
                            for k in range(3):
                                # k=0 center (halo offset +1), k=1 left (0), k=2 right (+2)
                                koff = (1, 0, 2)[k]
                                rhs = cur16[:, koff + t0 + n * 512: koff + t0 + n * 512 + 512]
                                lhsT = w1_sb[:, k * M1 + m * 128: k * M1 + (m + 1) * 128]
                                mm(outap, lhsT, rhs, k == 0, k == 2)
                    hh = wpool.tile([128, 2048], mm_dt, tag="h2")
                    nc.scalar.activation(hh[:], ps[:], AF.Gelu)
                    h2[r] = hh

                    if r % 2 == 1:
                        # GEMM2 for rules r-1, r
                        ps2 = ppool.tile([128, 2048], F32, tag="mm")
                        for rr, roff in ((r - 1, 0), (r, TT)):
                            for n in range(2):
                                outap = ps2[:, roff + n * 512: roff + (n + 1) * 512]
                                for k in range(2):
                                    rhs = h2[rr][:, k * TT + n * 512: k * TT + (n + 1) * 512]
                                    lhsT = w2_sb[:, k * R * D + rr * 128: k * R * D + (rr + 1) * 128]
                                    mm(outap, lhsT, rhs, k == 0, k == 1)
                        tt_ = wpool.tile([128, 2048], F32, tag="t2")
                        nc.scalar.activation(tt_[:], ps2[:], AF.Tanh)
                        if r == 1:
                            acc = apool.tile([128, TT], F32, tag="acc")
                            nc.vector.tensor_scalar_mul(acc[:], tt_[:, 0:TT], wp[0])
                        else:
                            nc.vector.scalar_tensor_tensor(
                                acc[:], tt_[:, 0:TT], wp[r - 1], acc[:], OP.mult, OP.add)
                        nc.vector.scalar_tensor_tensor(
                            acc[:], tt_[:, TT:2 * TT], wp[r], acc[:], OP.mult, OP.add)
                # new = alpha*old + acc
                nc.vector.scalar_tensor_tensor(
                    nxt[:, 1 + t0:1 + t0 + TT], cur[:, 1 + t0:1 + t0 + TT],
                    alpha, acc[:], OP.mult, OP.add)
                nc.scalar.copy(nxt16[:, 1 + t0:1 + t0 + TT],
                               nxt[:, 1 + t0:1 + t0 + TT])
            nc.vector.tensor_copy(nxt16[:, 0:1], nxt16[:, T:T + 1])
            nc.vector.tensor_copy(nxt16[:, T + 1:T + 2], nxt16[:, 1:2])
            cur, nxt = nxt, cur
            cur16, nxt16 = nxt16, cur16

        # ---- LayerNorm over D + store ----
        # Stage final cells through an ACT copy into the dead ping-pong
        # buffer: LN transposes then depend only on ACT (1 sync wait).
        stage = nxt
        for half in range(2):
            nc.scalar.copy(stage[:, 1 + half * 2048: 1 + (half + 1) * 2048],
                           cur[:, 1 + half * 2048: 1 + (half + 1) * 2048])
        xall = lnp.tile([128, T], F32, tag="xall")      # [token part, D] blocks
        for half in range(2):
            ps = ppool.tile([128, 2048], F32, tag="mm")
            # depend on the last column block of this half's stage copy so the
            # transposes' ACT wait is already covered by the absorber
            absorb_mm(ps, stage[:, 1 + half * 2048 + 2048 - 128: 1 + (half + 1) * 2048])
            for j in range(16):
                jj = half * 16 + j
                nc.tensor.transpose(
                    ps[:, j * 128:(j + 1) * 128],
                    stage[:, 1 + jj * 128: 1 + (jj + 1) * 128], ident[:])
            nc.vector.tensor_copy(xall[:, half * 2048:(half + 1) * 2048], ps[:])

        nblk = T // 128  # 32
        ssum = lnp.tile([128, nblk], F32, tag="ssum")
        ssq = lnp.tile([128, nblk], F32, tag="ssq")
        sq_scr = lnp.tile([128, 128], F32, tag="sqscr")
        for j in range(nblk):
            blk = xall[:, j * 128:(j + 1) * 128]
            nc.scalar.activation(sq_scr[:], blk, AF.Square,
                                 accum_out=ssq[:, j:j + 1])
            nc.vector.tensor_reduce(ssum[:, j:j + 1], blk,
                                    mybir.AxisListType.X, OP.add)
        mu = lnp.tile([128, nblk], F32, tag="mu")
        v = lnp.tile([128, nblk], F32, tag="v")
        scr = lnp.tile([128, nblk], F32, tag="scr")
        rstd = lnp.tile([128, nblk], F32, tag="rstd")
        nmr = lnp.tile([128, nblk], F32, tag="nmr")
        nc.vector.tensor_scalar_mul(mu[:], ssum[:], 1.0 / D)
        nc.vector.tensor_scalar_mul(v[:], ssq[:], 1.0 / D)
        nc.vector.tensor_mul(scr[:], mu[:], mu[:])
        nc.vector.tensor_sub(v[:], v[:], scr[:])
        nc.vector.tensor_scalar_add(v[:], v[:], LN_EPS)   # v = var + eps
        nc.scalar.sqrt(scr[:], v[:])                      # low-precision table
        nc.vector.reciprocal(rstd[:], scr[:])             # r0 ~ 1/sqrt(v)
        # one Newton step: r = r0 * (1.5 - 0.5*v*r0^2)
        nc.vector.tensor_mul(scr[:], rstd[:], rstd[:])
        nc.vector.tensor_mul(scr[:], scr[:], v[:])
        nc.vector.tensor_scalar(scr[:], scr[:], -0.5, 1.5, OP.mult, OP.add)
        nc.vector.tensor_mul(rstd[:], rstd[:], scr[:])
        nc.vector.scalar_tensor_tensor(nmr[:], mu[:], -1.0, rstd[:],
                                       OP.mult, OP.mult)
        for half in range(2):
            obig = lnp.tile([128, 2048], F32, tag="obig")
            for j in range(16):
                jj = half * 16 + j
                o = obig[:, j * 128:(j + 1) * 128]
                nc.vector.tensor_scalar(o, xall[:, jj * 128:(jj + 1) * 128],
                                        rstd[:, jj:jj + 1], nmr[:, jj:jj + 1],
                                        OP.mult, OP.add)
                if apply_gb:
                    nc.vector.tensor_mul(o, o, gb_sb[:, 0:D])
                    nc.vector.tensor_add(o, o, gb_sb[:, D:2 * D])
            dst = y_d[half * 2048:(half + 1) * 2048, :].rearrange(
                "(j p) d -> p j d", p=128)
            nc.sync.dma_start(dst, obig[:].rearrange("p (j d) -> p j d", j=16))
    nc.compile()
    return nc


def _prep_weights(inputs, dtype=np.float16):
    W1 = np.asarray(inputs["W1"], dtype)   # [R, 3D, 2D]
    W2 = np.asarray(inputs["W2"], dtype)   # [R, 2D, D]
    # w1[kk, k*M1 + r*H2 + h] = W1[r, k*128 + kk, h]
    w1 = np.ascontiguousarray(
        W1.reshape(R, 3, 128, H2).transpose(2, 1, 0, 3).reshape(128, 3 * M1))
    # w2[kk, k*R*D + r*D + d] = W2[r, k*128 + kk, d]
    w2 = np.ascontiguousarray(
        W2.reshape(R, 2, 128, D).transpose(2, 1, 0, 3).reshape(128, 2 * R * D))
    return w1, w2


def kernel(**inputs):
    rule_w, alpha, n_evolve = _selectors(inputs)
    b1 = np.asarray(inputs["b1"], np.float32)
    b2 = np.asarray(inputs["b2"], np.float32)
    assert not b1.any() and not b2.any(), "nonzero rule biases unsupported"
    ln_g = np.asarray(inputs["ln_g"], np.float32)
    ln_b = np.asarray(inputs["ln_b"], np.float32)
    apply_gb = bool((ln_g != 1.0).any() or ln_b.any())

    nc = build_nc(n_evolve, alpha, rule_w, apply_gb=apply_gb)

    w1, w2 = _prep_weights(inputs)
    x = np.asarray(inputs["cells_state"], np.float32)   # [B, T, D]
    in_maps = []
    for b in range(B):
        m = {"x": np.ascontiguousarray(x[b]), "w1": w1, "w2": w2}
        if apply_gb:
            m["gb"] = np.ascontiguousarray(
                np.stack([np.tile(ln_g, (128, 1)), np.tile(ln_b, (128, 1))]))
        in_maps.append(m)
    res = run_bass_kernel_spmd(nc, in_maps, list(range(B)))
    global LAST_RESULTS
    LAST_RESULTS = res
    out = np.stack([res.results[b]["y"] for b in range(B)])
    return out.astype(np.float32)

